# revision 1
# baseline (speedup 1.0000x reference)
"""Trainium2 Bass kernel for nn_Classifier (attention-pool + linear + classifier).

Reference math (per state n of 64):
    attn  = softmax(output_set @ states[n].T, axis=-1)      # [64io, 512s]
    mix   = attn @ states[n]                                # [64io, 1024h]
    o     = [mix | output_set] @ Wo + bo                    # [64io, 1024h]
    logit = tanh(o).flatten() @ Wc + bc                     # [64]

Sharding: data-parallel over the leading n_states dim — 8 states per core on
8 cores. Each core computes its own [8, 64] logits slice; host concatenates.

Per-core layout strategy:
  - states are processed in PAIRS packed into the 128-partition dim
    (state A on partitions 0-63, state B on 64-127), with col-tiled
    matmuls (tile_position) so the two M=64 matmuls use the full PE array.
  - statesT (h-major) is prepared host-side; attn contracts over h, mix
    contracts over s, so both orientations are needed on chip.
  - transposes between chained matmuls (attn->mix, mix->o, tanh->classifier)
    are 128x128 PE transposes.
  - const = output_set @ Wo[1024:] + bo is state-independent: computed once.
"""

import os
import sys

import numpy as np

for _p in ("/opt/trn_rl_repo",):
    if _p not in sys.path:
        sys.path.insert(0, _p)

import concourse.bass as bass
import concourse.mybir as mybir
import concourse.tile as tile
from concourse import bacc
from concourse.masks import make_identity

IO, H, S, NTOT = 64, 1024, 512, 64
NCORES = 8
NLOC = NTOT // NCORES  # states per core
P = 128
HC = H // P  # 8 h-chunks
SC = S // P  # 4 s-chunks
NPAIR = NLOC // 2

USE_BF16 = os.environ.get("KBASS_FP32", "0") != "1"

if USE_BF16:
    import ml_dtypes

    DT = mybir.dt.bfloat16
    NPDT = ml_dtypes.bfloat16
else:
    DT = mybir.dt.float32
    NPDT = np.float32

F32 = mybir.dt.float32
AX = mybir.AxisListType
AF = mybir.ActivationFunctionType

# SBUF buffer counts (DMA double-buffering across state pairs)
ST_BUFS = 4 if USE_BF16 else 2
SN_BUFS = 4 if USE_BF16 else 2


def build_bass(reps=1):
    nc = bacc.Bacc(
        "TRN2", target_bir_lowering=False, debug=False, num_devices=NCORES
    )

    statesT_d = nc.declare_dram_parameter("statesT", [NLOC, H, S], DT, isOutput=False)
    states_d = nc.declare_dram_parameter("states", [NLOC, S, H], DT, isOutput=False)
    osT2_d = nc.declare_dram_parameter("osT2", [H, 2 * IO], DT, isOutput=False)
    wo_top_d = nc.declare_dram_parameter("wo_top", [H, H], DT, isOutput=False)
    wo_bot_d = nc.declare_dram_parameter("wo_bot", [H, H], DT, isOutput=False)
    bo2_d = nc.declare_dram_parameter("bo2", [P, H], F32, isOutput=False)
    # classifier weights, pair-packed: [hp, j, hc, t*64+c] = Wc[(2j+t)*H + hc*128 + hp, c]
    wc_d = nc.declare_dram_parameter("wc", [P, IO // 2, HC, P], DT, isOutput=False)
    bct_d = nc.declare_dram_parameter("bct", [IO, NLOC], F32, isOutput=False)
    out_d = nc.declare_dram_parameter("logitsT", [IO, NLOC], F32, isOutput=True)

    with tile.TileContext(nc) as tc:
        with (
            tc.tile_pool(name="consts", bufs=1) as consts,
            tc.tile_pool(name="stT", bufs=ST_BUFS) as stT_pool,
            tc.tile_pool(name="sn", bufs=SN_BUFS) as sn_pool,
            tc.tile_pool(name="wstream", bufs=2) as wstream,
            tc.tile_pool(name="work", bufs=2) as work,
            tc.tile_pool(name="sm", bufs=4) as sm_pool,
            tc.tile_pool(name="ps_attn", bufs=2, space="PSUM") as ps_attn,
            tc.tile_pool(name="ps_tr", bufs=2, space="PSUM") as ps_tr,
            tc.tile_pool(name="ps_mix", bufs=1, space="PSUM") as ps_mix,
            tc.tile_pool(name="ps_o", bufs=1, space="PSUM") as ps_o,
        ):
            # ---- constants ----
            osT2_sb = consts.tile([P, HC, 2 * IO], DT)
            wo_top_sb = consts.tile([P, HC, H], DT)
            ident = consts.tile([P, P], DT)
            bo2_sb = consts.tile([P, H], F32)
            bct_sb = consts.tile([IO, NLOC], F32)
            const_sb = consts.tile([P, H], F32)
            # tanh(o) transposed, io-major: [hp, hc, io, state]
            tT_all = consts.tile([P, HC, IO, NLOC], DT)

            nc.sync.dma_start(osT2_sb[:], osT2_d.rearrange("(hc p) i -> p hc i", p=P))
            nc.sync.dma_start(
                wo_top_sb[:], wo_top_d.rearrange("(hc p) h -> p hc h", p=P)
            )
            nc.sync.dma_start(bo2_sb[:], bo2_d[:])
            nc.sync.dma_start(bct_sb[:], bct_d[:])
            make_identity(nc, ident[:])

            for _rep in range(reps):
                # ---- const = output_set @ Wo_bot + bo, duplicated on both halves ----
                wob_tiles = []
                for half in range(2):
                    wob = wstream.tile([P, HC // 2, H], DT, tag="wstream")
                    nc.sync.dma_start(
                        wob[:],
                        wo_bot_d[half * (H // 2) : (half + 1) * (H // 2), :].rearrange(
                            "(hc p) h -> p hc h", p=P
                        ),
                    )
                    wob_tiles.append(wob)
                cps = ps_o.tile([P, H], F32, tag="ps_o")
                for hc in range(HC):
                    wob = wob_tiles[hc // (HC // 2)]
                    for hh in range(2):
                        nc.tensor.matmul(
                            cps[:, hh * 512 : (hh + 1) * 512],
                            lhsT=osT2_sb[:, hc, :],
                            rhs=wob[:, hc % (HC // 2), hh * 512 : (hh + 1) * 512],
                            start=(hc == 0),
                            stop=(hc == HC - 1),
                        )
                # two single-wait DVE ops (walrus rejects TT instructions that
                # need >1 sync wait: copy joins PE, add joins the bo2 DMA)
                nc.vector.tensor_copy(const_sb[:], cps[:])
                nc.vector.tensor_add(const_sb[:], const_sb[:], bo2_sb[:])

                # ---- per state-pair pipeline ----
                for pi in range(NPAIR):
                    a, b = 2 * pi, 2 * pi + 1
                    stT = {}
                    sn = {}
                    for st in (a, b):
                        stT[st] = stT_pool.tile([P, HC, S], DT, tag="stT", name=f"stT_{st}")
                        nc.sync.dma_start(
                            stT[st][:], statesT_d[st].rearrange("(hc p) s -> p hc s", p=P)
                        )
                        sn[st] = sn_pool.tile([P, SC, H], DT, tag="sn", name=f"sn_{st}")
                        nc.sync.dma_start(
                            sn[st][:], states_d[st].rearrange("(sc p) h -> p sc h", p=P)
                        )

                    # attn scores: [128(ioA|ioB), 512s]
                    aps = ps_attn.tile([P, S], F32, tag="ps_attn")
                    for hc in range(HC):
                        for s_i, st in ((0, a), (1, b)):
                            nc.tensor.matmul(
                                aps[s_i * IO : (s_i + 1) * IO, :],
                                lhsT=osT2_sb[:, hc, s_i * IO : (s_i + 1) * IO],
                                rhs=stT[st][:, hc, :],
                                start=(hc == 0),
                                stop=(hc == HC - 1),
                                tile_position=(0, s_i * IO),
                                skip_group_check=True,
                            )

                    # softmax over s (free axis), both states at once
                    negmax = sm_pool.tile([P, 1], F32, tag="negmax")
                    nc.vector.reduce_max(negmax[:], aps[:], axis=AX.X, negate=True)
                    sumexp = sm_pool.tile([P, 1], F32, tag="sumexp")
                    exps = work.tile([P, S], F32, tag="exps")
                    # warm ACT's view of the DVE clock (negmax) so the Exp only
                    # carries a single PE sync wait
                    actw = sm_pool.tile([P, 1], F32, tag="actw")
                    nc.scalar.copy(actw[0:1, :], negmax[0:1, :])
                    nc.scalar.activation(
                        exps[:], aps[:], AF.Exp, bias=negmax[:], scale=1.0,
                        accum_out=sumexp[:],
                    )
                    rinv = sm_pool.tile([P, 1], F32, tag="rinv")
                    nc.vector.reciprocal(rinv[:], sumexp[:])
                    attn_w = work.tile([P, S], DT, tag="attn_w")
                    nc.vector.tensor_scalar_mul(attn_w[:], exps[:], rinv[:])

                    # attn^T via PE transposes: [128s, (ioA|ioB)]
                    atps = ps_tr.tile([P, 512], DT, tag="ps_tr")
                    for sc in range(SC):
                        nc.tensor.transpose(
                            atps[:, sc * P : (sc + 1) * P],
                            attn_w[:, sc * P : (sc + 1) * P],
                            ident[:],
                        )
                    attnT = work.tile([P, SC, P], DT, tag="attnT")
                    for sc in range(SC):
                        nc.vector.tensor_copy(
                            attnT[:, sc, :], atps[:, sc * P : (sc + 1) * P]
                        )

                    # mix = attn @ states: [128(ioA|ioB), 1024h]
                    mps = ps_mix.tile([P, H], F32, tag="ps_mix")
                    for sc in range(SC):
                        for s_i, st in ((0, a), (1, b)):
                            for hh in range(2):
                                nc.tensor.matmul(
                                    mps[s_i * IO : (s_i + 1) * IO, hh * 512 : (hh + 1) * 512],
                                    lhsT=attnT[:, sc, s_i * IO : (s_i + 1) * IO],
                                    rhs=sn[st][:, sc, hh * 512 : (hh + 1) * 512],
                                    start=(sc == 0),
                                    stop=(sc == SC - 1),
                                    tile_position=(0, s_i * IO),
                                skip_group_check=True,
                                )
                    mix_sb = work.tile([P, H], DT, tag="mix_sb")
                    nc.vector.tensor_copy(mix_sb[:], mps[:])

                    # mix^T via PE transposes: [128h, (ioA|ioB)] per h-chunk
                    mtps = [ps_tr.tile([P, 512], DT, tag="ps_tr", name=f"mtps_{j}") for j in range(2)]
                    for hc in range(HC):
                        nc.tensor.transpose(
                            mtps[hc // 4][:, (hc % 4) * P : (hc % 4 + 1) * P],
                            mix_sb[:, hc * P : (hc + 1) * P],
                            ident[:],
                        )
                    mixT = work.tile([P, HC, P], DT, tag="mixT")
                    for hc in range(HC):
                        nc.vector.tensor_copy(
                            mixT[:, hc, :], mtps[hc // 4][:, (hc % 4) * P : (hc % 4 + 1) * P]
                        )

                    # o = mix @ Wo_top (+const later): [128(ioA|ioB), 1024h]
                    ops_ = ps_o.tile([P, H], F32, tag="ps_o")
                    for hc in range(HC):
                        for s_i in (0, 1):
                            for hh in range(2):
                                nc.tensor.matmul(
                                    ops_[s_i * IO : (s_i + 1) * IO, hh * 512 : (hh + 1) * 512],
                                    lhsT=mixT[:, hc, s_i * IO : (s_i + 1) * IO],
                                    rhs=wo_top_sb[:, hc, hh * 512 : (hh + 1) * 512],
                                    start=(hc == 0),
                                    stop=(hc == HC - 1),
                                    tile_position=(0, s_i * IO),
                                skip_group_check=True,
                                )
                    osum = work.tile([P, H], F32, tag="osum")
                    nc.vector.tensor_add(osum[:], ops_[:], const_sb[:])
                    t_sb = work.tile([P, H], DT, tag="t_sb")
                    nc.scalar.activation(t_sb[:], osum[:], AF.Tanh)

                    # t^T into the shared classifier operand buffer
                    ttps = [ps_tr.tile([P, 512], DT, tag="ps_tr", name=f"ttps_{j}") for j in range(2)]
                    for hc in range(HC):
                        nc.tensor.transpose(
                            ttps[hc // 4][:, (hc % 4) * P : (hc % 4 + 1) * P],
                            t_sb[:, hc * P : (hc + 1) * P],
                            ident[:],
                        )
                    for hc in range(HC):
                        # transpose-out cols are (state, io); tT_all wants (io, state)
                        src = ttps[hc // 4][:, (hc % 4) * P : (hc % 4 + 1) * P]
                        nc.vector.tensor_copy(
                            tT_all[:, hc, :, 2 * pi : 2 * pi + 2],
                            src.rearrange("p (st io) -> p io st", st=2),
                        )

                # ---- classifier, i-pair packed (valid quadrants disjoint in PSUM):
                # lhsT = [Wc_{2j} | Wc_{2j+1}] (128 cols -> FWL), rhs = [t_{2j} | t_{2j+1}]
                # psum rows 0:64 accumulate even-i partial logitsT, 64:128 odd-i.
                lgps = ps_attn.tile([P, 2 * NLOC], F32, tag="ps_attn", name="lgps")
                NJG = 8  # i-pairs per streamed Wc group
                for jg in range((IO // 2) // NJG):
                    wcg = wstream.tile([P, NJG, HC, P], DT, tag="wstream")
                    nc.sync.dma_start(wcg[:], wc_d[:, jg * NJG : (jg + 1) * NJG])
                    for jl in range(NJG):
                        j = jg * NJG + jl
                        for hc in range(HC):
                            nc.tensor.matmul(
                                lgps[:],
                                lhsT=wcg[:, jl, hc, :],
                                rhs=tT_all[:, hc, 2 * j : 2 * j + 2, :],
                                start=(j == 0 and hc == 0),
                                stop=(j == IO // 2 - 1 and hc == HC - 1),
                                skip_group_check=True,
                            )
                # epilogue: logitsT = q_even + q_odd + bc  (each DVE op: <=1 wait)
                lt_sb = work.tile([P, NLOC], F32, tag="lt_sb")
                nc.vector.tensor_copy(lt_sb[0:IO, :], lgps[0:IO, 0:NLOC])
                nc.vector.tensor_copy(lt_sb[IO:P, :], lgps[IO:P, NLOC : 2 * NLOC])
                nc.vector.tensor_add(lt_sb[0:IO, :], lt_sb[0:IO, :], bct_sb[:])
                # fold the odd-i half onto the even half (cross-partition: DMA accum)
                nc.gpsimd.dma_start(
                    lt_sb[0:IO, :], lt_sb[IO:P, :], accum_op=mybir.AluOpType.add
                )
                nc.sync.dma_start(out_d[:], lt_sb[0:IO, :])

    nc.compile()
    return nc


def make_in_maps(states, output_set, Wo, bo, Wc, bc):
    """Build the per-core input maps (host-side sharding + layout prep)."""
    states = np.asarray(states, dtype=np.float32)
    output_set = np.asarray(output_set, dtype=np.float32)
    Wo = np.asarray(Wo, dtype=np.float32)
    bo = np.asarray(bo, dtype=np.float32)
    Wc = np.asarray(Wc, dtype=np.float32)
    bc = np.asarray(bc, dtype=np.float32)

    osT = output_set.T  # [H, IO]
    shared = {
        "osT2": np.ascontiguousarray(np.concatenate([osT, osT], axis=1)).astype(NPDT),
        "wo_top": np.ascontiguousarray(Wo[:H]).astype(NPDT),
        "wo_bot": np.ascontiguousarray(Wo[H:]).astype(NPDT),
        "bo2": np.ascontiguousarray(np.tile(bo, (P, 1))).astype(np.float32),
        # Wc[(2j+t)*H + hc*128 + hp, c] -> [hp, j, hc, t*64+c]
        "wc": np.ascontiguousarray(
            Wc.reshape(IO // 2, 2, HC, P, IO)
            .transpose(3, 0, 2, 1, 4)
            .reshape(P, IO // 2, HC, P)
        ).astype(NPDT),
        "bct": np.ascontiguousarray(np.tile(bc[:, None], (1, NLOC))).astype(
            np.float32
        ),
    }
    in_maps = []
    for k in range(NCORES):
        sl = states[k * NLOC : (k + 1) * NLOC]  # [NLOC, S, H]
        in_maps.append(
            {
                "states": np.ascontiguousarray(sl).astype(NPDT),
                "statesT": np.ascontiguousarray(sl.transpose(0, 2, 1)).astype(NPDT),
                **shared,
            }
        )
    return in_maps


_NC_CACHE = {}


def get_nc(reps=1):
    if reps not in _NC_CACHE:
        _NC_CACHE[reps] = build_bass(reps)
    return _NC_CACHE[reps]


def kernel(states, output_set, Wo, bo, Wc, bc):
    from concourse.bass_utils import run_bass_kernel_spmd

    nc = get_nc()
    in_maps = make_in_maps(states, output_set, Wo, bo, Wc, bc)
    res = run_bass_kernel_spmd(nc, in_maps, core_ids=list(range(NCORES)))
    out = np.concatenate(
        [np.asarray(res.results[k]["logitsT"]).T for k in range(NCORES)], axis=0
    )
    return out.astype(np.float32)

